# revision 1
# baseline (speedup 1.0000x reference)
"""AttentionBlock kernel for Trainium2 (Bass/Tile), data-parallel over batch.

Reference computation (per batch b of 8, N = H*W = 4096, C = 256):
    q = x @ wq + bq ; k = x @ wk + bk ; v = x @ wv + bv          [N, C]
    s = (q @ k^T) / sqrt(C)                                      [N, N]
    a = softmax(s, axis=-1)
    o = a @ v                                                    [N, C]
    out = x + o @ wp + bp                                        [N, C]

Sharding: one batch per NeuronCore (8 batches, 8 cores), no collectives.

Per-core layout strategy ("S^T layout" — no attention transposes):
  - x is loaded naturally [n, c] and PE-transposed once to xT [c, n].
  - qT, kT [c, n] computed with weights as stationary operands.
  - v [n, c] computed naturally (xT slices stationary).
  - For each query block of 512 columns:
      for each key chunk m (32 chunks of 128 rows):
        sT[m-chunk]   = kT-slice.T @ qT-block      (PSUM [128k, 512q])
        eT = exp(sT / 16)                          (ACT, PSUM->SBUF)
        rawT[c-chunk] += v-slice.T @ eT            (PSUM [128c, 512q], accum)
        colsum += eT                               (DVE, SBUF accum)
      denom[q-sub] = colsum-slice.T @ ones         (partition reduce, [128q, 1])
      recip = 1/denom                              (DVE)
      proj[q-sub] = rawT-slice.T @ wp              (natural [128q, 256c])
      out = x + proj * recip + (bp + bv @ wp)      (DVE epilogue)
  The softmax denominator division is deferred: it commutes with the wp
  contraction because it is a per-query scaling. bv also commutes through
  (attention rows sum to 1), folded into an effective output bias.

All matmuls run as float32r (tf32-like, 1 col/cycle at N>=256; plain fp32
is 4x slower). All data stays fp32 in SBUF.
"""

import numpy as np

import concourse.mybir as mybir
import concourse.tile as tile
from concourse import bacc
from concourse import bass_utils
from concourse.masks import make_identity

# Problem shape (hardcoded per contract).
B, H, W, C = 8, 64, 64, 256
N = H * W  # 4096
P = 128
C2 = C // P  # 2 chunks of input/output channels
NK = N // P  # 32 key chunks
QB = 512  # query block width (free dim of S^T matmuls)
NQB = N // QB  # 8 query blocks
QSUB = QB // P  # 4 query sub-blocks of 128 per block
SCALE = float(C) ** -0.5  # 1/16

F32 = mybir.dt.float32
F32R = mybir.dt.float32r
AF = mybir.ActivationFunctionType

_CACHED_NC = None


def _build(repeat=1, rep_xt=1, rep_qkv=1, rep_attn=1):
    nc = bacc.Bacc("TRN2", target_bir_lowering=False, debug=False)

    x_d = nc.dram_tensor("x", [N, C], F32, kind="ExternalInput").ap()
    w_d = {
        name: nc.dram_tensor(name, [C, C], F32, kind="ExternalInput").ap()
        for name in ("wq", "wk", "wv", "wp")
    }
    b_d = {
        name: nc.dram_tensor(name, [C], F32, kind="ExternalInput").ap()
        for name in ("bq", "bk", "bv", "bp")
    }
    out_d = nc.dram_tensor("out", [N, C], F32, kind="ExternalOutput").ap()

    with tile.TileContext(nc) as tc:
        for _ in range(repeat):
            _emit(nc, tc, x_d, w_d, b_d, out_d, rep_xt, rep_qkv, rep_attn)
    nc.compile()
    return nc


def _emit(nc, tc, x_d, w_d, b_d, out_d, rep_xt=1, rep_qkv=1, rep_attn=1):
    import contextlib

    ctx = contextlib.ExitStack()
    with ctx:
        consts = ctx.enter_context(tc.tile_pool(name="consts", bufs=1))
        big = ctx.enter_context(tc.tile_pool(name="big", bufs=1))
        xload = ctx.enter_context(tc.tile_pool(name="xload", bufs=4))
        exp_pool = ctx.enter_context(tc.tile_pool(name="exp", bufs=4))
        sums = ctx.enter_context(tc.tile_pool(name="sums", bufs=2))
        rawsb = ctx.enter_context(tc.tile_pool(name="rawsb", bufs=2))
        epil = ctx.enter_context(tc.tile_pool(name="epil", bufs=4))

        # PSUM budget is 8 banks of [128, 2KB]. Tags within a pool each get
        # their own `bufs` slots, so everything in ps_proj/ps_misc shares one
        # tag: 2 (st) + 2 (raw) + 2 (proj) + 2 (misc) = 8 banks.
        ps_st = ctx.enter_context(tc.tile_pool(name="ps_st", bufs=2, space="PSUM"))
        ps_raw = ctx.enter_context(tc.tile_pool(name="ps_raw", bufs=1, space="PSUM"))
        ps_proj = ctx.enter_context(tc.tile_pool(name="ps_proj", bufs=2, space="PSUM"))
        ps_misc = ctx.enter_context(tc.tile_pool(name="ps_misc", bufs=2, space="PSUM"))

        # ---- constants -------------------------------------------------
        identity = consts.tile([P, P], F32)
        make_identity(nc, identity[:])

        # Tiny matmuls (denominator reduce, bias prep) run in plain fp32:
        # fp32r has ISA restrictions at small moving dims (N=1 is invalid).
        ones_col = consts.tile([P, 1], F32)
        nc.vector.memset(ones_col[:], 1.0)

        # Weights: [C, C] -> [P, C2, C] (ci = c2*128 + p on partitions).
        w_sb = {}
        for name in ("wq", "wk", "wv", "wp"):
            w_sb[name] = consts.tile([P, C2, C], F32R, tag=f"w_{name}", name=f"w_{name}")
            nc.sync.dma_start(
                w_sb[name][:],
                w_d[name].rearrange("(c2 p) co -> p c2 co", p=P).bitcast(F32R),
            )
        # Plain-fp32 copy of wp for the (tiny) bias-prep matmul.
        wp_f32 = consts.tile([P, C2, C], F32)
        nc.sync.dma_start(wp_f32[:], w_d["wp"].rearrange("(c2 p) co -> p c2 co", p=P))
        # bq, bk as per-partition scalars in the [co] layout: [P, C2].
        bqk_sb = {}
        for name in ("bq", "bk"):
            bqk_sb[name] = consts.tile([P, C2], F32, tag=f"b_{name}", name=f"b_{name}")
            nc.sync.dma_start(
                bqk_sb[name][:], b_d[name].rearrange("(c2 p) -> p c2", p=P)
            )
        # bv, bp as [1, C] rows (plain fp32 — the bias prep matmuls are tiny).
        bv_row = consts.tile([1, C], F32)
        bp_row = consts.tile([1, C], F32)
        nc.sync.dma_start(bv_row[:], b_d["bv"][None, :])
        nc.sync.dma_start(bp_row[:], b_d["bp"][None, :])

        # bp_eff[co] = bp[co] + sum_c bv[c] wp[c, co]; broadcast to [P, C].
        # Transpose bv_row to a column via matmul (K=1): bv_col = bv_row.T.
        bv_colps = ps_misc.tile([P, C2, 1], F32, tag="misc")
        for c2 in range(C2):
            # [1, 128] slice -> [128, 1]
            nc.tensor.matmul(
                bv_colps[:, c2],
                bv_row[:, c2 * P : (c2 + 1) * P],
                ones_col[:1],
                start=True,
                stop=True,
            )
        bv_col = consts.tile([P, C2, 1], F32)
        nc.vector.tensor_copy(bv_col[:], bv_colps[:])
        # bvwp[1, C] = sum_c2 bv_col[:, c2].T @ wp[:, c2, :]
        bvwp_ps = ps_misc.tile([1, C], F32, tag="misc")
        for c2 in range(C2):
            nc.tensor.matmul(
                bvwp_ps[:],
                bv_col[:, c2],
                wp_f32[:, c2, :],
                start=(c2 == 0),
                stop=(c2 == C2 - 1),
            )
        bp_eff_row = consts.tile([1, C], F32)
        nc.vector.tensor_add(bp_eff_row[:], bvwp_ps[:], bp_row[:])
        # Broadcast to all partitions: ones_col @ bp_eff_row.
        ones_row = consts.tile([1, P], F32)
        nc.vector.memset(ones_row[:], 1.0)
        bpb_ps = ps_misc.tile([P, C], F32, tag="misc")
        nc.tensor.matmul(bpb_ps[:], ones_row[:], bp_eff_row[:], start=True, stop=True)
        bp_bcast = consts.tile([P, C], F32)
        nc.vector.tensor_copy(bp_bcast[:], bpb_ps[:])

        # ---- xT: [P, C2, N] ------------------------------------------
        xT = big.tile([P, C2, N], F32R, tag="xT")
        for _ in range(rep_xt):
         for nk in range(NK):
            x_tile = xload.tile([P, C], F32, tag="x_in")
            nc.sync.dma_start(x_tile[:], x_d[nk * P : (nk + 1) * P, :])
            for c2 in range(C2):
                tps = ps_misc.tile([P, P], F32, tag="misc")
                nc.tensor.transpose(
                    tps[:], x_tile[:, c2 * P : (c2 + 1) * P], identity[:]
                )
                nc.scalar.copy(xT[:, c2, nk * P : (nk + 1) * P], tps[:])

        # ---- qT, kT: [P, C2, N]; v: [P, NK, C] -----------------------
        qT = big.tile([P, C2, N], F32R, tag="qT")
        kT = big.tile([P, C2, N], F32R, tag="kT")
        for _ in range(rep_qkv):
         for dst, wname, bname in ((qT, "wq", "bq"), (kT, "wk", "bk")):
            for co2 in range(C2):
                for nb in range(NQB):
                    pst = ps_proj.tile([P, QB], F32, tag="mm_out")
                    for ci2 in range(C2):
                        nc.tensor.matmul(
                            pst[:],
                            w_sb[wname][:, ci2, co2 * P : (co2 + 1) * P],
                            xT[:, ci2, nb * QB : (nb + 1) * QB],
                            start=(ci2 == 0),
                            stop=(ci2 == C2 - 1),
                        )
                    # copy + bias (per-partition co bias)
                    nc.scalar.activation(
                        dst[:, co2, nb * QB : (nb + 1) * QB],
                        pst[:],
                        AF.Identity,
                        bias=bqk_sb[bname][:, co2 : co2 + 1],
                    )

        v_sb = big.tile([P, NK, C], F32R, tag="v")
        for _ in range(rep_qkv):
         for nk in range(NK):
            pst = ps_proj.tile([P, C], F32, tag="mm_out")
            for ci2 in range(C2):
                nc.tensor.matmul(
                    pst[:],
                    xT[:, ci2, nk * P : (nk + 1) * P],
                    w_sb["wv"][:, ci2, :],
                    start=(ci2 == 0),
                    stop=(ci2 == C2 - 1),
                )
            # v = psum + bv (broadcast via bv_col trick is per-free-dim; use
            # row-broadcast tile) — bv enters through bp_eff instead, so v is
            # the *raw* x@wv here. (bv commutes: attn rows sum to 1.)
            nc.vector.tensor_copy(v_sb[:, nk, :], pst[:])

        # ---- attention over query blocks ------------------------------
        for _ in range(rep_attn):
         for qb in range(NQB):
            qslice = slice(qb * QB, (qb + 1) * QB)
            rawT_ps = ps_raw.tile([P, C2, QB], F32, tag="rawT")
            colsum = sums.tile([P, QB], F32, tag="colsum")

            for mk in range(NK):
                st_ps = ps_st.tile([P, QB], F32, tag="st")
                for ci2 in range(C2):
                    nc.tensor.matmul(
                        st_ps[:],
                        kT[:, ci2, mk * P : (mk + 1) * P],
                        qT[:, ci2, qslice],
                        start=(ci2 == 0),
                        stop=(ci2 == C2 - 1),
                    )
                e_t = exp_pool.tile([P, QB], F32R, tag="eT")
                nc.scalar.activation(e_t[:], st_ps[:], AF.Exp, scale=SCALE)
                # accumulate raw output (transposed)
                for c2 in range(C2):
                    nc.tensor.matmul(
                        rawT_ps[:, c2],
                        v_sb[:, mk, c2 * P : (c2 + 1) * P],
                        e_t[:],
                        start=(mk == 0),
                        stop=(mk == NK - 1),
                    )
                # accumulate softmax denominators
                if mk == 0:
                    nc.vector.tensor_copy(colsum[:], e_t[:])
                else:
                    nc.vector.tensor_add(colsum[:], colsum[:], e_t[:])

            # copy rawT to SBUF (fp32r: feeds the proj matmul)
            rawT_sb = rawsb.tile([P, C2, QB], F32R, tag="rawT_sb")
            nc.scalar.copy(rawT_sb[:, 0], rawT_ps[:, 0])
            nc.vector.tensor_copy(rawT_sb[:, 1], rawT_ps[:, 1])

            # denominators: [128q, 1] per q-sub via ones reduction (plain
            # fp32 matmul — N=1 is invalid for fp32r, and cost is trivial)
            den_ps = ps_misc.tile([P, QSUB], F32, tag="misc")
            for qs in range(QSUB):
                nc.tensor.matmul(
                    den_ps[:, qs : qs + 1],
                    colsum[:, qs * P : (qs + 1) * P],
                    ones_col[:],
                    start=True,
                    stop=True,
                )
            recip = sums.tile([P, QSUB], F32, tag="recip")
            nc.vector.reciprocal(recip[:], den_ps[:])

            # proj + epilogue per q-sub
            for qs in range(QSUB):
                pj_ps = ps_proj.tile([P, C], F32, tag="mm_out")
                for c2 in range(C2):
                    nc.tensor.matmul(
                        pj_ps[:],
                        rawT_sb[:, c2, qs * P : (qs + 1) * P],
                        w_sb["wp"][:, c2, :],
                        start=(c2 == 0),
                        stop=(c2 == C2 - 1),
                    )
                n0 = qb * QB + qs * P
                x_res = epil.tile([P, C], F32, tag="x_res")
                nc.sync.dma_start(x_res[:], x_d[n0 : n0 + P, :])
                o_t = epil.tile([P, C], F32, tag="o_t")
                # o = proj * recip[q]  (per-partition scalar)
                nc.vector.tensor_scalar_mul(o_t[:], pj_ps[:], recip[:, qs : qs + 1])
                # o += bp_eff (broadcast row)
                nc.vector.tensor_add(o_t[:], o_t[:], bp_bcast[:])
                # o += x residual
                nc.vector.tensor_add(o_t[:], o_t[:], x_res[:])
                nc.sync.dma_start(out_d[n0 : n0 + P, :], o_t[:])


def kernel(**inputs):
    global _CACHED_NC
    if _CACHED_NC is None:
        _CACHED_NC = _build()
    nc = _CACHED_NC

    x = np.ascontiguousarray(inputs["x"], dtype=np.float32)  # [B, H, W, C]
    shared = {
        name: np.ascontiguousarray(inputs[name], dtype=np.float32)
        for name in ("wq", "bq", "wk", "bk", "wv", "bv", "wp", "bp")
    }
    in_maps = [
        {"x": x[b].reshape(N, C), **shared} for b in range(B)
    ]
    res = bass_utils.run_bass_kernel_spmd(nc, in_maps, core_ids=list(range(B)))
    out = np.stack([res.results[b]["out"] for b in range(B)], axis=0)
    return out.reshape(B, H, W, C)

